# revision 6
# baseline (speedup 1.0000x reference)
"""Linear attention (non-causal, elu+1 feature map) on 8 Trainium2 cores.

Math per (batch b, head h), with phi(x) = elu(x)+1:
    C_aug = phi(K)^T @ [V | 1]        # (64, 65): context (64x64) + k_sum col
    numer = phi(Q) @ C_aug[:, :64]    # (T, 64)
    denom = phi(Q) @ C_aug[:, 64]     # (T,)
    out   = numer / denom             # eps=1e-6 is negligible vs denom ~1e5

Sharding: 16 heads / 8 cores = 2 heads per core, all 4 batches per core
(per-head problems are fully independent). Host pre-transposes Q per core
to (e, t) layout so every device matmul contracts along SBUF partitions
with zero on-device transposes, packs [K | V | 1] per head into one
tensor (one DMA per head -> one DMA-queue wait per matmul; the ISA allows
only 2 sync waits on a weight-load), and the ones column makes k_sum fall
out of matmul1 for free.

Device layouts (per core, all f32, all DMA-contiguous):
    qt:  (4, 128, 4096)    qt[b, hh*64+e, t] = Q[b, t, head(hh)*64+e]
    kva: (4, 2, 4096, 129) [K | V | 1] per head
    o:   (4, 2, 4096, 64)  natural per-head output

t-blocking: kva tiles assign t = p*32 + n (partition p, tile n) so each
DMA reads ~16KB contiguous per partition; matmul2 uses strided lhsT
column chunks (t = j*32 + n) so output blocks land contiguous in HBM too.
The t->(partition, tile) assignment is sum-invariant for matmul1 and
self-consistent for matmul2's output indexing.
"""

from contextlib import ExitStack

import numpy as np

import concourse.bacc as bacc
import concourse.bass as bass
import concourse.mybir as mybir
import concourse.tile as tile
from concourse.bass_utils import run_bass_kernel_spmd

B = 4
T = 4096
D = 1024
H = 16
E = 64
EA = E + 1
W = E + EA  # 129 cols per kva row
NCORES = 8
HPC = H // NCORES  # 2 heads per core
P = 128
NT = T // P  # 32 t-tiles
GRP = 4  # matmul2 chunks per psum group
DT = mybir.dt.float32
AF = mybir.ActivationFunctionType
ALU = mybir.AluOpType


def _phi(nc, x, tmp):
    """x <- elu(x)+1 == max(x+1, exp(min(x, 0))), tmp as scratch.

    x and tmp may be multi-dim APs of matching shape.
    """
    nc.vector.tensor_scalar_min(tmp, x, 0.0)
    nc.scalar.activation(tmp, tmp, AF.Exp)
    nc.vector.scalar_tensor_tensor(x, x, 1.0, tmp, ALU.add, ALU.max)


def build_nc():
    nc = bacc.Bacc("TRN2", target_bir_lowering=False, debug=False)
    qt = nc.dram_tensor("qt", [B, P, T], DT, kind="ExternalInput").ap()
    kva = nc.dram_tensor("kva", [B, HPC, T, W], DT, kind="ExternalInput").ap()
    o = nc.dram_tensor("o", [B, HPC, T, E], DT, kind="ExternalOutput").ap()

    with tile.TileContext(nc) as tc, ExitStack() as ctx:
        qt_pool = ctx.enter_context(tc.tile_pool(name="qt", bufs=2))
        mq_pool = ctx.enter_context(tc.tile_pool(name="mq", bufs=2))
        kva_pool = ctx.enter_context(tc.tile_pool(name="kva", bufs=3))
        mk_pool = ctx.enter_context(tc.tile_pool(name="mk", bufs=2))
        c_pool = ctx.enter_context(tc.tile_pool(name="c", bufs=2))
        r_pool = ctx.enter_context(tc.tile_pool(name="r", bufs=4))
        out_pool = ctx.enter_context(tc.tile_pool(name="out", bufs=2))
        psc_pool = ctx.enter_context(tc.tile_pool(name="psc", bufs=2, space="PSUM"))
        pso_pool = ctx.enter_context(tc.tile_pool(name="pso", bufs=4, space="PSUM"))

        for b in range(B):
            # Q^T for both heads: (128, 4096), partition = hh*64+e
            qt_t = qt_pool.tile([P, T], DT)
            nc.sync.dma_start(qt_t[:], qt[b])
            mq = mq_pool.tile([P, T], DT)
            _phi(nc, qt_t[:], mq[:])

            # ---- matmul1: C_aug[e, m] = sum_t phiK[t, e] * [V|1][t, m] ----
            # head 0 -> psum partitions 0:64, head 1 -> 64:128 (col tiling)
            psum_c = psc_pool.tile([P, EA], DT)
            for h in range(HPC):
                kva_t = kva_pool.tile([P, NT * W], DT)
                nc.sync.dma_start(
                    kva_t[:].rearrange("p (n e) -> p n e", e=W),
                    kva[b, h].rearrange("(p n) e -> p n e", p=P),
                )
                k3 = kva_t[:].rearrange("p (n e) -> p n e", e=W)[:, :, 0:E]
                mk = mk_pool.tile([P, NT * E], DT)
                _phi(nc, k3, mk[:].rearrange("p (n e) -> p n e", e=E))
                for n in range(NT):
                    nc.tensor.matmul(
                        psum_c[h * E : (h + 1) * E, :],
                        lhsT=kva_t[:, n * W : n * W + E],
                        rhs=kva_t[:, n * W + E : (n + 1) * W],
                        start=(n == 0),
                        stop=(n == NT - 1),
                        tile_position=(0, h * E),
                    )
            c_sb = c_pool.tile([P, EA], DT)
            nc.vector.tensor_copy(c_sb[:], psum_c[:])

            # ---- matmul2 + normalize: out[t, d] = phiQ[t,:] @ C[:, d] / denom[t]
            for h in range(HPC):
                out_sb = out_pool.tile([P, NT * E], DT)
                for g in range(NT // GRP):
                    ps_o = pso_pool.tile([P, GRP * EA], DT)
                    for j in range(GRP):
                        n = g * GRP + j
                        nc.tensor.matmul(
                            ps_o[:, j * EA : (j + 1) * EA],
                            lhsT=qt_t[h * E : (h + 1) * E, n::NT],
                            rhs=c_sb[h * E : (h + 1) * E, :],
                            start=True,
                            stop=True,
                        )
                    r_sb = r_pool.tile([P, GRP], DT)
                    nc.vector.reciprocal(r_sb[:], ps_o[:, E::EA])
                    for j in range(GRP):
                        n = g * GRP + j
                        nc.vector.tensor_scalar_mul(
                            out_sb[:, n * E : (n + 1) * E],
                            ps_o[:, j * EA : j * EA + E],
                            r_sb[:, j : j + 1],
                        )
                nc.sync.dma_start(
                    o[b, h].rearrange("(p n) e -> p n e", p=P),
                    out_sb[:].rearrange("p (n e) -> p n e", e=E),
                )
    nc.finalize()
    return nc


_NC_CACHE = None


def _get_nc():
    global _NC_CACHE
    if _NC_CACHE is None:
        _NC_CACHE = build_nc()
    return _NC_CACHE


def make_in_maps(query, key, value):
    query = np.ascontiguousarray(query, dtype=np.float32)
    key = np.ascontiguousarray(key, dtype=np.float32)
    value = np.ascontiguousarray(value, dtype=np.float32)
    in_maps = []
    for c in range(NCORES):
        lo = c * HPC * E
        hi = lo + HPC * E
        qt = np.ascontiguousarray(query[:, :, lo:hi].transpose(0, 2, 1))
        kva = np.empty((B, HPC, T, W), np.float32)
        kva[..., :E] = key[:, :, lo:hi].reshape(B, T, HPC, E).transpose(0, 2, 1, 3)
        kva[..., E : E + E] = (
            value[:, :, lo:hi].reshape(B, T, HPC, E).transpose(0, 2, 1, 3)
        )
        kva[..., E + E] = 1.0
        in_maps.append({"qt": qt, "kva": kva})
    return in_maps


def assemble_out(results):
    out = np.empty((B, T, D), np.float32)
    for c in range(NCORES):
        oc = results[c]["o"]  # (B, HPC, T, E)
        out[:, :, c * HPC * E : (c + 1) * HPC * E] = oc.transpose(0, 2, 1, 3).reshape(
            B, T, HPC * E
        )
    return out


def run(query, key, value, **spmd_kwargs):
    nc = _get_nc()
    in_maps = make_in_maps(query, key, value)
    res = run_bass_kernel_spmd(nc, in_maps, core_ids=list(range(NCORES)), **spmd_kwargs)
    return assemble_out(res.results), res


def kernel(query, key, value):
    out, _ = run(query, key, value)
    return out


# revision 10
# speedup vs baseline: 1.0021x; 1.0021x over previous
"""Linear attention (non-causal, elu+1 feature map) on 8 Trainium2 cores.

Math per (batch b, head h), with phi(x) = elu(x)+1:
    C_aug = phi(K)^T @ [V | 1]        # (64, 65): context (64x64) + k_sum col
    numer = phi(Q) @ C_aug[:, :64]    # (T, 64)
    denom = phi(Q) @ C_aug[:, 64]     # (T,)
    out   = numer / denom             # eps=1e-6 is negligible vs denom ~1e5

Sharding: 16 heads / 8 cores = 2 heads per core, all 4 batches per core
(per-head problems are fully independent). Host pre-transposes Q per core
to (e, t) layout so every device matmul contracts along SBUF partitions
with zero on-device transposes, packs [K | V | 1] per head into one
tensor (one DMA per head -> one DMA-queue wait per matmul; the ISA allows
only 2 sync waits on a weight-load), and the ones column makes k_sum fall
out of matmul1 for free.

Device layouts (per core, all f32, all DMA-contiguous):
    qt:  (4, 128, 4096)    qt[b, hh*64+e, t] = Q[b, t, head(hh)*64+e]
    kva: (4, 2, 4096, 129) [K | V | 1] per head
    o:   (4, 2, 4096, 64)  natural per-head output

t-blocking: kva tiles assign t = p*32 + n (partition p, tile n) so each
DMA reads ~16KB contiguous per partition; matmul2 uses strided lhsT
column chunks (t = j*32 + n) so output blocks land contiguous in HBM too.
The t->(partition, tile) assignment is sum-invariant for matmul1 and
self-consistent for matmul2's output indexing.
"""

from contextlib import ExitStack

import numpy as np

import concourse.bacc as bacc
import concourse.bass as bass
import concourse.mybir as mybir
import concourse.tile as tile
from concourse.bass_utils import run_bass_kernel_spmd

B = 4
T = 4096
D = 1024
H = 16
E = 64
EA = E + 1
W = E + EA  # 129 cols per kva row
NCORES = 8
HPC = H // NCORES  # 2 heads per core
P = 128
NT = T // P  # 32 t-tiles
GRP = 4  # matmul2 chunks per psum group
DT = mybir.dt.float32
AF = mybir.ActivationFunctionType
ALU = mybir.AluOpType


def _phi(nc, x, tmp):
    """x <- elu(x)+1 == max(x+1, exp(min(x, 0))), tmp as scratch.

    x and tmp may be multi-dim APs of matching shape.
    """
    nc.vector.tensor_scalar_min(tmp, x, 0.0)
    nc.scalar.activation(tmp, tmp, AF.Exp)
    nc.vector.scalar_tensor_tensor(x, x, 1.0, tmp, ALU.add, ALU.max)


def build_nc():
    nc = bacc.Bacc("TRN2", target_bir_lowering=False, debug=False)
    qt = nc.dram_tensor("qt", [B, P, T], DT, kind="ExternalInput").ap()
    kva = nc.dram_tensor("kva", [B, HPC, T, W], DT, kind="ExternalInput").ap()
    o = nc.dram_tensor("o", [B, HPC, T, E], DT, kind="ExternalOutput").ap()

    with tile.TileContext(nc) as tc, ExitStack() as ctx:
        qt_pool = ctx.enter_context(tc.tile_pool(name="qt", bufs=2))
        mq_pool = ctx.enter_context(tc.tile_pool(name="mq", bufs=2))
        kva_pool = ctx.enter_context(tc.tile_pool(name="kva", bufs=3))
        mk_pool = ctx.enter_context(tc.tile_pool(name="mk", bufs=2))
        c_pool = ctx.enter_context(tc.tile_pool(name="c", bufs=2))
        r_pool = ctx.enter_context(tc.tile_pool(name="r", bufs=4))
        out_pool = ctx.enter_context(tc.tile_pool(name="out", bufs=2))
        psc_pool = ctx.enter_context(tc.tile_pool(name="psc", bufs=2, space="PSUM"))
        pso_pool = ctx.enter_context(tc.tile_pool(name="pso", bufs=4, space="PSUM"))

        for b in range(B):
            # Q^T for both heads: (128, 4096), partition = hh*64+e
            qt_t = qt_pool.tile([P, T], DT)
            nc.sync.dma_start(qt_t[:], qt[b])
            mq = mq_pool.tile([P, T], DT)
            _phi(nc, qt_t[:], mq[:])

            # ---- matmul1: C_aug[e, m] = sum_t phiK[t, e] * [V|1][t, m] ----
            # head 0 -> psum partitions 0:64, head 1 -> 64:128 (col tiling)
            psum_c = psc_pool.tile([P, EA], DT)
            for h in range(HPC):
                kva_t = kva_pool.tile([P, NT * W], DT)
                nc.sync.dma_start(
                    kva_t[:].rearrange("p (n e) -> p n e", e=W),
                    kva[b, h].rearrange("(p n) e -> p n e", p=P),
                )
                k3 = kva_t[:].rearrange("p (n e) -> p n e", e=W)[:, :, 0:E]
                mk = mk_pool.tile([P, NT * E], DT)
                _phi(nc, k3, mk[:].rearrange("p (n e) -> p n e", e=E))
                for n in range(NT):
                    nc.tensor.matmul(
                        psum_c[h * E : (h + 1) * E, :],
                        lhsT=kva_t[:, n * W : n * W + E],
                        rhs=kva_t[:, n * W + E : (n + 1) * W],
                        start=(n == 0),
                        stop=(n == NT - 1),
                        tile_position=(0, h * E),
                    )
            c_sb = c_pool.tile([P, EA], DT)
            nc.vector.tensor_copy(c_sb[:], psum_c[:])

            # ---- matmul2 + normalize: out[t, d] = phiQ[t,:] @ C[:, d] / denom[t]
            for h in range(HPC):
                out_sb = out_pool.tile([P, NT * E], DT)
                for g in range(NT // GRP):
                    ps_o = pso_pool.tile([P, GRP * EA], DT)
                    for j in range(GRP):
                        n = g * GRP + j
                        nc.tensor.matmul(
                            ps_o[:, j * EA : (j + 1) * EA],
                            lhsT=qt_t[h * E : (h + 1) * E, n::NT],
                            rhs=c_sb[h * E : (h + 1) * E, :],
                            start=True,
                            stop=True,
                        )
                    r_sb = r_pool.tile([P, GRP], DT)
                    nc.vector.reciprocal(r_sb[:], ps_o[:, E::EA])
                    for j in range(GRP):
                        n = g * GRP + j
                        nc.vector.tensor_scalar_mul(
                            out_sb[:, n * E : (n + 1) * E],
                            ps_o[:, j * EA : j * EA + E],
                            r_sb[:, j : j + 1],
                        )
                nc.sync.dma_start(
                    o[b, h].rearrange("(p n) e -> p n e", p=P),
                    out_sb[:].rearrange("p (n e) -> p n e", e=E),
                )
    nc.finalize()
    return nc


_NC_CACHE = None


def _get_nc():
    global _NC_CACHE
    if _NC_CACHE is None:
        _NC_CACHE = build_nc()
    return _NC_CACHE


def make_in_maps(query, key, value):
    query = np.ascontiguousarray(query, dtype=np.float32)
    key = np.ascontiguousarray(key, dtype=np.float32)
    value = np.ascontiguousarray(value, dtype=np.float32)
    in_maps = []
    for c in range(NCORES):
        lo = c * HPC * E
        hi = lo + HPC * E
        qt = np.ascontiguousarray(query[:, :, lo:hi].transpose(0, 2, 1))
        kva = np.empty((B, HPC, T, W), np.float32)
        kva[..., :E] = key[:, :, lo:hi].reshape(B, T, HPC, E).transpose(0, 2, 1, 3)
        kva[..., E : E + E] = (
            value[:, :, lo:hi].reshape(B, T, HPC, E).transpose(0, 2, 1, 3)
        )
        kva[..., E + E] = 1.0
        in_maps.append({"qt": qt, "kva": kva})
    return in_maps


def assemble_out(results):
    out = np.empty((B, T, D), np.float32)
    for c in range(NCORES):
        oc = results[c]["o"]  # (B, HPC, T, E)
        out[:, :, c * HPC * E : (c + 1) * HPC * E] = oc.transpose(0, 2, 1, 3).reshape(
            B, T, HPC * E
        )
    return out


def run(query, key, value, **spmd_kwargs):
    nc = _get_nc()
    in_maps = make_in_maps(query, key, value)
    res = run_bass_kernel_spmd(nc, in_maps, core_ids=list(range(NCORES)), **spmd_kwargs)
    return assemble_out(res.results), res


def kernel(query, key, value):
    out, _ = run(query, key, value)
    return out
